# revision 13
# baseline (speedup 1.0000x reference)
"""AlternateTimelineGenerator Trainium2 kernel.

Data-parallel over the batch: 16384 rows -> 8 NeuronCores x 2048 rows.
Per core, everything runs in "transposed" layout (feature on SBUF
partitions, batch rows on the free dimension) so the whole chain of
matmuls composes without on-chip transposes; only the two inputs are
transposed once at the start via TensorE. Outputs are written transposed
and de-transposed on the host during the gather.

Per step (rows chunked in CH-column slices):
  gates0 = W_hh0 @ h0 + W_ih0 @ s            (PE, bf16, PSUM fp32)
  i,f,g,o = ACT sigmoid/tanh (+bias)          -> bf16 SBUF
  c0 = f*c0 + i*g; h0 = o*tanh(c0)            (DVE, c fp32 / h bf16)
  gates1 = W_ih1 @ h0 + W_hh1 @ h1            ... same cell for layer 1
  rom  = relu(W_om1 @ [h1; s])                (PE + ACT relu, bf16)
  mod  = tanh(W_om2 @ rom + b)                (PE + ACT, bf16)
  cur += 0.1 * mod                            (DVE scalar_tensor_tensor)
  prob = sigmoid(W_pe2 @ relu(W_pe1 @ cur))   (PE fp32r + DVE + ACT)
"""

import numpy as np
import ml_dtypes

B, FDIM, H, E = 16384, 512, 256, 128
NCORES = 8
R = B // NCORES        # rows per core
CH = 1024              # rows per chunk
P = 128

_BUILD_CACHE = {}


def _build(num_steps: int, R: int = R, CH: int = CH, stage: int = 99):
    import concourse.bass as bass
    import concourse.tile as tile
    from concourse import bacc, mybir
    from concourse.masks import make_identity

    NCH = R // CH
    MM = min(CH, 512)  # matmul moving-dim tile
    NMM = CH // MM
    dt = mybir.dt
    AF = mybir.ActivationFunctionType
    ALU = mybir.AluOpType
    f32, f32r, bf = dt.float32, dt.float32r, dt.bfloat16

    nc = bacc.Bacc("TRN2", target_bir_lowering=False, debug=False)

    # ---- DRAM I/O ----
    base_d = nc.dram_tensor("base", [R, FDIM], f32, kind="ExternalInput").ap()
    cf_d = nc.dram_tensor("cf", [R, FDIM], f32, kind="ExternalInput").ap()
    wt_g0_d = nc.dram_tensor("wt_g0", [3 * P, 4 * H], bf, kind="ExternalInput").ap()
    wt_g1_d = nc.dram_tensor("wt_g1", [4 * P, 4 * H], bf, kind="ExternalInput").ap()
    wt_om1_d = nc.dram_tensor("wt_om1", [3 * P, 256], bf, kind="ExternalInput").ap()
    wt_om2_d = nc.dram_tensor("wt_om2", [256, FDIM], bf, kind="ExternalInput").ap()
    wt_se1_d = nc.dram_tensor("wt_se1", [FDIM, 256], f32, kind="ExternalInput").ap()
    wt_se2_d = nc.dram_tensor("wt_se2", [256, E], f32, kind="ExternalInput").ap()
    wt_pe1_d = nc.dram_tensor("wt_pe1", [FDIM, E], f32, kind="ExternalInput").ap()
    wt_pe2_d = nc.dram_tensor("wt_pe2", [E, 1], f32, kind="ExternalInput").ap()
    bg0_d = nc.dram_tensor("bg0", [4 * H], f32, kind="ExternalInput").ap()
    bg1_d = nc.dram_tensor("bg1", [4 * H], f32, kind="ExternalInput").ap()
    bse1_d = nc.dram_tensor("b_se1", [256], f32, kind="ExternalInput").ap()
    bse2_d = nc.dram_tensor("b_se2", [E], f32, kind="ExternalInput").ap()
    bom1_d = nc.dram_tensor("b_om1", [256], f32, kind="ExternalInput").ap()
    bom2_d = nc.dram_tensor("b_om2", [FDIM], f32, kind="ExternalInput").ap()
    bpe1_d = nc.dram_tensor("b_pe1", [E], f32, kind="ExternalInput").ap()
    bpe2_d = nc.dram_tensor("b_pe2", [1], f32, kind="ExternalInput").ap()

    alt_d = nc.dram_tensor("alt_t", [num_steps, FDIM, R], f32, kind="ExternalOutput").ap()
    probs_d = nc.dram_tensor("probs_t", [num_steps, R], f32, kind="ExternalOutput").ap()
    s_d = nc.dram_tensor("s_t", [E, R], f32, kind="ExternalOutput").ap()

    with tile.TileContext(nc) as tc:
        with (
            tc.tile_pool(name="consts", bufs=1) as consts,
            tc.tile_pool(name="state", bufs=1) as state,
            tc.tile_pool(name="psum", bufs=4, space="PSUM") as psum,
        ):
            # ---- weights / biases to SBUF ----
            wt_g0 = consts.tile([P, 3, 4 * H], bf, tag="wt_g0")
            nc.sync.dma_start(wt_g0[:], wt_g0_d.rearrange("(k p) m -> p k m", p=P))
            wt_g1 = consts.tile([P, 4, 4 * H], bf, tag="wt_g1")
            nc.sync.dma_start(wt_g1[:], wt_g1_d.rearrange("(k p) m -> p k m", p=P))
            wt_om1 = consts.tile([P, 3, 256], bf, tag="wt_om1")
            nc.sync.dma_start(wt_om1[:], wt_om1_d.rearrange("(k p) m -> p k m", p=P))
            wt_om2 = consts.tile([P, 2, FDIM], bf, tag="wt_om2")
            nc.sync.dma_start(wt_om2[:], wt_om2_d.rearrange("(k p) m -> p k m", p=P))
            wt_se1 = consts.tile([P, 4, 256], f32, tag="wt_se1")
            nc.sync.dma_start(wt_se1[:], wt_se1_d.rearrange("(k p) m -> p k m", p=P))
            wt_se2 = consts.tile([P, 2, E], f32, tag="wt_se2")
            nc.sync.dma_start(wt_se2[:], wt_se2_d.rearrange("(k p) m -> p k m", p=P))
            wt_pe1 = consts.tile([P, 4, E], f32, tag="wt_pe1")
            nc.sync.dma_start(wt_pe1[:], wt_pe1_d.rearrange("(k p) m -> p k m", p=P))
            wt_pe2 = consts.tile([P, 1], f32, tag="wt_pe2")
            nc.sync.dma_start(wt_pe2[:], wt_pe2_d)

            def bias_tile(name, dram, m):
                tl = consts.tile([P, m], f32, tag=name)
                nc.sync.dma_start(tl[:], dram.rearrange("(m p) -> p m", p=P))
                return tl

            bg0 = bias_tile("bg0", bg0_d, 8)
            bg1 = bias_tile("bg1", bg1_d, 8)
            bse1 = bias_tile("bse1", bse1_d, 2)
            bse2 = bias_tile("bse2", bse2_d, 1)
            bom1 = bias_tile("bom1", bom1_d, 2)
            bom2 = bias_tile("bom2", bom2_d, 4)
            bpe1 = bias_tile("bpe1", bpe1_d, 1)
            bpe2 = consts.tile([1, 1], f32, tag="bpe2")
            nc.sync.dma_start(bpe2[:], bpe2_d.rearrange("m -> m ()"))

            ident = consts.tile([P, P], f32, tag="ident")
            make_identity(nc, ident[:])

            # ---- persistent state ----
            h0 = state.tile([P, 2, R], bf, tag="h0")
            h1 = state.tile([P, 2, R], bf, tag="h1")
            c0 = state.tile([P, 2, R], f32, tag="c0")
            c1 = state.tile([P, 2, R], f32, tag="c1")
            cur = state.tile([P, 4, R], f32, tag="cur")
            s = state.tile([P, R], f32, tag="s")
            sbf = state.tile([P, R], bf, tag="sbf")
            for tl in (h0, h1, c0, c1):
                nc.vector.memset(tl[:], 0.0)

            # ---- transpose inputs (cf -> cfT, base -> cur) + encoder ----
            with tc.tile_pool(name="setup", bufs=3) as setup, \
                 tc.tile_pool(name="setup1", bufs=1) as setup1:
                cfT = setup1.tile([P, 4, R], f32, tag="cfT")
                r1 = setup1.tile([P, 2, R], f32, tag="r1")
                for src, dst in ((cf_d, cfT), (base_d, cur)):
                    for rt in range(R // P):
                        tmp = setup.tile([P, FDIM], f32, tag="tr_in")
                        nc.sync.dma_start(tmp[:], src[rt * P:(rt + 1) * P, :])
                        pt = psum.tile([P, 4, P], f32, tag="ps")
                        for ft in range(4):
                            nc.tensor.transpose(
                                pt[:, ft, :], tmp[:, ft * P:(ft + 1) * P], ident[:]
                            )
                        nc.vector.tensor_copy(dst[:, :, rt * P:(rt + 1) * P], pt[:])

                for n in range(NCH):
                    for m in range(2):
                        ps = psum.tile([P, CH], f32, tag="ps")
                        for h2 in range(NMM):
                            lo = n * CH + h2 * MM
                            for k in range(4):
                                nc.tensor.matmul(
                                    ps[:, h2 * MM:(h2 + 1) * MM],
                                    wt_se1[:, k, m * P:(m + 1) * P],
                                    cfT[:, k, lo:lo + MM],
                                    start=(k == 0), stop=(k == 3),
                                )
                        nc.scalar.activation(
                            r1[:, m, n * CH:(n + 1) * CH], ps[:], AF.Relu,
                            bias=bse1[:, m:m + 1],
                        )
                    ps = psum.tile([P, CH], f32, tag="ps")
                    for h2 in range(NMM):
                        lo = n * CH + h2 * MM
                        for k in range(2):
                            nc.tensor.matmul(
                                ps[:, h2 * MM:(h2 + 1) * MM],
                                wt_se2[:, k, :],
                                r1[:, k, lo:lo + MM],
                                start=(k == 0), stop=(k == 1),
                            )
                    nc.scalar.activation(
                        s[:, n * CH:(n + 1) * CH], ps[:], AF.Identity,
                        bias=bse2[:, 0:1],
                    )
                nc.vector.tensor_copy(sbf[:], s[:])
                nc.sync.dma_start(s_d[:, :], s[:])

            # ---- recurrence ----
            with tc.tile_pool(name="loop", bufs=2) as loop:
                for t in range(num_steps * (stage >= 1)):
                    for n in range(NCH):
                        cs = slice(n * CH, (n + 1) * CH)

                        for wt, sg_tag, bgt, rhs_of_k, nk, cc, hh in (
                            (wt_g0, "sg0", bg0,
                             lambda k, lo, hi: h0[:, k, lo:hi] if k < 2 else sbf[:, lo:hi],
                             3, c0, h0),
                            (wt_g1, "sg1", bg1,
                             lambda k, lo, hi: h0[:, k, lo:hi] if k < 2 else h1[:, k - 2, lo:hi],
                             4, c1, h1),
                        ):
                            sg = loop.tile([P, 8, CH], bf, tag="sg")
                            for m in range(8):
                                ps = psum.tile([P, CH], f32, tag="ps")
                                for h2 in range(NMM):
                                    lo = n * CH + h2 * MM
                                    for k in range(nk):
                                        nc.tensor.matmul(
                                            ps[:, h2 * MM:(h2 + 1) * MM],
                                            wt[:, k, m * P:(m + 1) * P],
                                            rhs_of_k(k, lo, lo + MM),
                                            start=(k == 0), stop=(k == nk - 1),
                                        )
                                func = AF.Tanh if m in (4, 5) else AF.Sigmoid
                                nc.scalar.activation(
                                    sg[:, m, :], ps[:], func, bias=bgt[:, m:m + 1]
                                )
                            # LSTM cell elementwise
                            nc.vector.tensor_tensor(
                                sg[:, 0:2, :], sg[:, 0:2, :], sg[:, 4:6, :], ALU.mult)
                            nc.vector.tensor_tensor(
                                cc[:, :, cs], sg[:, 2:4, :], cc[:, :, cs], ALU.mult)
                            nc.vector.tensor_tensor(
                                cc[:, :, cs], cc[:, :, cs], sg[:, 0:2, :], ALU.add)
                            nc.scalar.activation(sg[:, 4:6, :], cc[:, :, cs], AF.Tanh)
                            nc.vector.tensor_tensor(
                                hh[:, :, cs], sg[:, 6:8, :], sg[:, 4:6, :], ALU.mult)

                        # outcome modifier
                        if stage < 2:
                            continue
                        rom = loop.tile([P, 2, CH], bf, tag="rom")
                        for m in range(2):
                            ps = psum.tile([P, CH], f32, tag="ps")
                            for h2 in range(NMM):
                                lo = n * CH + h2 * MM
                                for k in range(3):
                                    rhs = (h1[:, k, lo:lo + MM] if k < 2
                                           else sbf[:, lo:lo + MM])
                                    nc.tensor.matmul(
                                        ps[:, h2 * MM:(h2 + 1) * MM],
                                        wt_om1[:, k, m * P:(m + 1) * P], rhs,
                                        start=(k == 0), stop=(k == 2),
                                    )
                            nc.scalar.activation(
                                rom[:, m, :], ps[:], AF.Relu, bias=bom1[:, m:m + 1])
                        mod = loop.tile([P, 4, CH], bf, tag="mod")
                        for m in range(4):
                            ps = psum.tile([P, CH], f32, tag="ps")
                            for h2 in range(NMM):
                                for k in range(2):
                                    nc.tensor.matmul(
                                        ps[:, h2 * MM:(h2 + 1) * MM],
                                        wt_om2[:, k, m * P:(m + 1) * P],
                                        rom[:, k, h2 * MM:(h2 + 1) * MM],
                                        start=(k == 0), stop=(k == 1),
                                    )
                            nc.scalar.activation(
                                mod[:, m, :], ps[:], AF.Tanh, bias=bom2[:, m:m + 1])
                        nc.vector.scalar_tensor_tensor(
                            cur[:, :, cs], mod[:], 0.1, cur[:, :, cs],
                            ALU.mult, ALU.add)
                        nc.sync.dma_start(
                            alt_d[t].rearrange("(fc p) r -> p fc r", p=P)[:, :, cs],
                            cur[:, :, cs],
                        )

                        # probability estimator
                        if stage < 3:
                            continue
                        ps = psum.tile([P, CH], f32, tag="ps")
                        for h2 in range(NMM):
                            lo = n * CH + h2 * MM
                            for k in range(4):
                                nc.tensor.matmul(
                                    ps[:, h2 * MM:(h2 + 1) * MM],
                                    wt_pe1[:, k, :],
                                    cur[:, k, lo:lo + MM],
                                    start=(k == 0), stop=(k == 3),
                                )
                        rp = loop.tile([P, CH], f32, tag="rp")
                        nc.vector.tensor_scalar(
                            rp[:], ps[:], bpe1[:, 0:1], 0.0, ALU.add, ALU.max)
                        if stage < 4:
                            continue
                        ps2 = psum.tile([P, CH], f32, tag="ps")
                        for h2 in range(NMM):
                            nc.tensor.matmul(
                                ps2[0:1, h2 * MM:(h2 + 1) * MM],
                                wt_pe2[:, 0:1],
                                rp[:, h2 * MM:(h2 + 1) * MM],
                                start=True, stop=True,
                            )
                        if stage < 5:
                            continue
                        pb = loop.tile([1, CH], f32, tag="pb")
                        nc.scalar.activation(
                            pb[0:1, :], ps2[0:1, :], AF.Sigmoid, bias=bpe2[0:1, 0:1])
                        if stage >= 6:
                            nc.sync.dma_start(
                                probs_d[t:t + 1, cs], pb[0:1, :])

    nc.compile()
    return nc


def _get_nc(num_steps: int):
    if num_steps not in _BUILD_CACHE:
        _BUILD_CACHE[num_steps] = _build(num_steps)
    return _BUILD_CACHE[num_steps]


def make_in_maps(inputs, R_=None):
    """Shard inputs + preprocess params into per-core input maps."""
    def npf(x):
        return np.asarray(x, np.float32)

    bf16 = ml_dtypes.bfloat16
    params = {
        "wt_g0": np.ascontiguousarray(
            np.concatenate([npf(inputs["w_hh0"]).T, npf(inputs["w_ih0"]).T], 0)
        ).astype(bf16),
        "wt_g1": np.ascontiguousarray(
            np.concatenate([npf(inputs["w_ih1"]).T, npf(inputs["w_hh1"]).T], 0)
        ).astype(bf16),
        "wt_om1": np.ascontiguousarray(npf(inputs["w_om1"]).T).astype(bf16),
        "wt_om2": np.ascontiguousarray(npf(inputs["w_om2"]).T).astype(bf16),
        "wt_se1": np.ascontiguousarray(npf(inputs["w_se1"]).T),
        "wt_se2": np.ascontiguousarray(npf(inputs["w_se2"]).T),
        "wt_pe1": np.ascontiguousarray(npf(inputs["w_pe1"]).T),
        "wt_pe2": np.ascontiguousarray(npf(inputs["w_pe2"]).T),
        "bg0": npf(inputs["b_ih0"]) + npf(inputs["b_hh0"]),
        "bg1": npf(inputs["b_ih1"]) + npf(inputs["b_hh1"]),
        "b_se1": npf(inputs["b_se1"]),
        "b_se2": npf(inputs["b_se2"]),
        "b_om1": npf(inputs["b_om1"]),
        "b_om2": npf(inputs["b_om2"]),
        "b_pe1": npf(inputs["b_pe1"]),
        "b_pe2": npf(inputs["b_pe2"]),
    }
    rr = R_ or R
    base = npf(inputs["base_timeline"])
    cf = npf(inputs["counterfactual_scenario"])
    ncores = base.shape[0] // rr
    in_maps = []
    for i in range(ncores):
        m = dict(params)
        m["base"] = np.ascontiguousarray(base[i * rr:(i + 1) * rr])
        m["cf"] = np.ascontiguousarray(cf[i * rr:(i + 1) * rr])
        in_maps.append(m)
    return in_maps


def assemble(results):
    """Gather per-core transposed outputs into full reference-shaped arrays."""
    alt = np.concatenate(
        [r["alt_t"].transpose(2, 0, 1) for r in results], axis=0)      # [B, T, 512]
    probs = np.concatenate(
        [r["probs_t"].T[:, :, None] for r in results], axis=0)         # [B, T, 1]
    s = np.concatenate([r["s_t"].T for r in results], axis=0)          # [B, 128]
    final = np.ascontiguousarray(alt[:, -1, :])                        # [B, 512]
    return alt, probs, s, final


def kernel(**inputs):
    from concourse.bass_utils import run_bass_kernel_spmd

    num_steps = int(np.asarray(inputs["num_steps"]))
    nc = _get_nc(num_steps)
    in_maps = make_in_maps(inputs)
    res = run_bass_kernel_spmd(nc, in_maps, core_ids=list(range(NCORES))).results
    return assemble(res)


# revision 14
# speedup vs baseline: 1.0595x; 1.0595x over previous
"""AlternateTimelineGenerator Trainium2 kernel.

Data-parallel over the batch: 16384 rows -> 8 NeuronCores x 2048 rows.
Per core, everything runs in "transposed" layout (feature on SBUF
partitions, batch rows on the free dimension) so the whole chain of
matmuls composes without on-chip transposes; only the two inputs are
transposed once at the start via TensorE. Outputs are written transposed
and de-transposed on the host during the gather.

Per step (rows chunked in CH-column slices):
  gates0 = W_hh0 @ h0 + W_ih0 @ s            (PE bf16, PSUM fp32)
  i,f,g,o = ACT sigmoid/tanh (+bias)          -> bf16 SBUF
  c0 = f*c0 + i*g; h0 = o*tanh(c0)            (DVE, c fp32 / h bf16)
  gates1 = W_ih1 @ h0 + W_hh1 @ h1            ... same cell for layer 1
  rom  = relu(W_om1 @ [h1; s])                (PE + DVE relu, bf16)
  mod  = tanh(W_om2 @ rom + b)                (PE + ACT, bf16)
  cur += 0.1 * mod                            (DVE scalar_tensor_tensor)
  P   += (0.1*W_pe1) @ mod                    (PE bf16; P = W_pe1@cur + b)
  prob = sigmoid(W_pe2 @ relu(P))             (PE bf16 + DVE + ACT)

The probability head keeps P = W_pe1 @ cur^T + b_pe1 as a recurrent
fp32 state so it never has to read the freshly-updated cur (which
would serialize PE behind the big DVE update each step).
"""

import numpy as np
import ml_dtypes

B, FDIM, H, E = 16384, 512, 256, 128
NCORES = 8
R = B // NCORES        # rows per core
CH = 512               # rows per chunk
P = 128

_BUILD_CACHE = {}


def _build(num_steps: int, R: int = R, CH: int = CH, stage: int = 99):
    import concourse.bass as bass
    import concourse.tile as tile
    from concourse import bacc, mybir
    from concourse.masks import make_identity

    NCH = R // CH
    MM = min(CH, 512)  # matmul moving-dim tile
    NMM = CH // MM
    dt = mybir.dt
    AF = mybir.ActivationFunctionType
    ALU = mybir.AluOpType
    f32, bf = dt.float32, dt.bfloat16

    nc = bacc.Bacc("TRN2", target_bir_lowering=False, debug=False)

    # ---- DRAM I/O ----
    base_d = nc.dram_tensor("base", [R, FDIM], f32, kind="ExternalInput").ap()
    cf_d = nc.dram_tensor("cf", [R, FDIM], f32, kind="ExternalInput").ap()
    wt_g0_d = nc.dram_tensor("wt_g0", [3 * P, 4 * H], bf, kind="ExternalInput").ap()
    wt_g1_d = nc.dram_tensor("wt_g1", [4 * P, 4 * H], bf, kind="ExternalInput").ap()
    wt_om1_d = nc.dram_tensor("wt_om1", [3 * P, 256], bf, kind="ExternalInput").ap()
    wt_om2_d = nc.dram_tensor("wt_om2", [256, FDIM], bf, kind="ExternalInput").ap()
    wt_se1_d = nc.dram_tensor("wt_se1", [FDIM, 256], bf, kind="ExternalInput").ap()
    wt_se2_d = nc.dram_tensor("wt_se2", [256, E], bf, kind="ExternalInput").ap()
    wt_pe1_d = nc.dram_tensor("wt_pe1", [FDIM, E], f32, kind="ExternalInput").ap()
    wt_pe1s_d = nc.dram_tensor("wt_pe1s", [FDIM, E], bf, kind="ExternalInput").ap()
    wt_pe2_d = nc.dram_tensor("wt_pe2", [E, 1], bf, kind="ExternalInput").ap()
    bg0_d = nc.dram_tensor("bg0", [4 * H], f32, kind="ExternalInput").ap()
    bg1_d = nc.dram_tensor("bg1", [4 * H], f32, kind="ExternalInput").ap()
    bse1_d = nc.dram_tensor("b_se1", [256], f32, kind="ExternalInput").ap()
    bse2_d = nc.dram_tensor("b_se2", [E], f32, kind="ExternalInput").ap()
    bom1_d = nc.dram_tensor("b_om1", [256], f32, kind="ExternalInput").ap()
    bom2_d = nc.dram_tensor("b_om2", [FDIM], f32, kind="ExternalInput").ap()
    bpe1_d = nc.dram_tensor("b_pe1", [E], f32, kind="ExternalInput").ap()
    bpe2_d = nc.dram_tensor("b_pe2", [1], f32, kind="ExternalInput").ap()

    alt_d = nc.dram_tensor("alt_t", [num_steps, FDIM, R], f32, kind="ExternalOutput").ap()
    probs_d = nc.dram_tensor("probs_t", [num_steps, R], f32, kind="ExternalOutput").ap()
    s_d = nc.dram_tensor("s_t", [E, R], f32, kind="ExternalOutput").ap()

    with tile.TileContext(nc) as tc:
        with (
            tc.tile_pool(name="consts", bufs=1) as consts,
            tc.tile_pool(name="state", bufs=1) as state,
            tc.tile_pool(name="psum", bufs=8, space="PSUM") as psum,
        ):
            # ---- weights / biases to SBUF ----
            def w_tile(name, dram, kk, mm_, dtype):
                tl = consts.tile([P, kk, mm_], dtype, tag=name)
                nc.sync.dma_start(tl[:], dram.rearrange("(k p) m -> p k m", p=P))
                return tl

            wt_g0 = w_tile("wt_g0", wt_g0_d, 3, 4 * H, bf)
            wt_g1 = w_tile("wt_g1", wt_g1_d, 4, 4 * H, bf)
            wt_om1 = w_tile("wt_om1", wt_om1_d, 3, 256, bf)
            wt_om2 = w_tile("wt_om2", wt_om2_d, 2, FDIM, bf)
            wt_se1 = w_tile("wt_se1", wt_se1_d, 4, 256, bf)
            wt_se2 = w_tile("wt_se2", wt_se2_d, 2, E, bf)
            wt_pe1 = w_tile("wt_pe1", wt_pe1_d, 4, E, f32)
            wt_pe1s = w_tile("wt_pe1s", wt_pe1s_d, 4, E, bf)
            wt_pe2 = consts.tile([P, 1], bf, tag="wt_pe2")
            nc.sync.dma_start(wt_pe2[:], wt_pe2_d)

            def bias_tile(name, dram, m):
                tl = consts.tile([P, m], f32, tag=name)
                nc.sync.dma_start(tl[:], dram.rearrange("(m p) -> p m", p=P))
                return tl

            bg0 = bias_tile("bg0", bg0_d, 8)
            bg1 = bias_tile("bg1", bg1_d, 8)
            bse1 = bias_tile("bse1", bse1_d, 2)
            bse2 = bias_tile("bse2", bse2_d, 1)
            bom1 = bias_tile("bom1", bom1_d, 2)
            bom2 = bias_tile("bom2", bom2_d, 4)
            bpe1 = bias_tile("bpe1", bpe1_d, 1)
            bpe2 = consts.tile([1, 1], f32, tag="bpe2")
            nc.sync.dma_start(bpe2[:], bpe2_d.rearrange("m -> m ()"))

            ident = consts.tile([P, P], f32, tag="ident")
            make_identity(nc, ident[:])

            # ---- persistent state ----
            h0 = state.tile([P, 2, R], bf, tag="h0")
            h1 = state.tile([P, 2, R], bf, tag="h1")
            c0 = state.tile([P, 2, R], f32, tag="c0")
            c1 = state.tile([P, 2, R], f32, tag="c1")
            cur = state.tile([P, 4, R], f32, tag="cur")
            pacc = state.tile([P, R], f32, tag="pacc")
            s = state.tile([P, R], f32, tag="s")
            sbf = state.tile([P, R], bf, tag="sbf")
            for tl in (h0, h1, c0, c1):
                nc.vector.memset(tl[:], 0.0)

            # ---- transpose inputs (cf -> cfT bf16, base -> cur fp32) ----
            with tc.tile_pool(name="setup", bufs=3) as setup, \
                 tc.tile_pool(name="setup1", bufs=1) as setup1:
                cfT = setup1.tile([P, 4, R], bf, tag="cfT")
                r1 = setup1.tile([P, 2, R], bf, tag="r1")
                for src, dst in ((cf_d, cfT), (base_d, cur)):
                    for rt in range(R // P):
                        tmp = setup.tile([P, FDIM], f32, tag="tr_in")
                        nc.sync.dma_start(tmp[:], src[rt * P:(rt + 1) * P, :])
                        pt = psum.tile([P, 4, P], f32, tag="ps")
                        for ft in range(4):
                            nc.tensor.transpose(
                                pt[:, ft, :], tmp[:, ft * P:(ft + 1) * P], ident[:]
                            )
                        nc.vector.tensor_copy(dst[:, :, rt * P:(rt + 1) * P], pt[:])

                # ---- scenario encoder (bf16) + P0 = W_pe1 @ base^T + b ----
                for n in range(R // 512):
                    sl = slice(n * 512, (n + 1) * 512)
                    for m in range(2):
                        ps = psum.tile([P, 512], f32, tag="ps")
                        for k in range(4):
                            nc.tensor.matmul(
                                ps[:], wt_se1[:, k, m * P:(m + 1) * P],
                                cfT[:, k, sl], start=(k == 0), stop=(k == 3),
                            )
                        nc.scalar.activation(
                            r1[:, m, sl], ps[:], AF.Relu, bias=bse1[:, m:m + 1])
                    ps = psum.tile([P, 512], f32, tag="ps")
                    for k in range(2):
                        nc.tensor.matmul(
                            ps[:], wt_se2[:, k, :], r1[:, k, sl],
                            start=(k == 0), stop=(k == 1),
                        )
                    nc.scalar.activation(
                        s[:, sl], ps[:], AF.Identity, bias=bse2[:, 0:1])
                    ps = psum.tile([P, 512], f32, tag="ps")
                    for k in range(4):
                        nc.tensor.matmul(
                            ps[:], wt_pe1[:, k, :], cur[:, k, sl],
                            start=(k == 0), stop=(k == 3),
                        )
                    nc.vector.tensor_scalar(
                        pacc[:, sl], ps[:], bpe1[:, 0:1], None, ALU.add)
                nc.vector.tensor_copy(sbf[:], s[:])
                nc.sync.dma_start(s_d[:, :], s[:])

            # ---- recurrence ----
            with tc.tile_pool(name="loop", bufs=2) as loop:
                for t in range(num_steps * (stage >= 1)):
                    for n in range(NCH):
                        cs = slice(n * CH, (n + 1) * CH)

                        for wt, bgt, rhs_of_k, nk, cc, hh in (
                            (wt_g0, bg0,
                             lambda k, lo, hi: h0[:, k, lo:hi] if k < 2 else sbf[:, lo:hi],
                             3, c0, h0),
                            (wt_g1, bg1,
                             lambda k, lo, hi: h0[:, k, lo:hi] if k < 2 else h1[:, k - 2, lo:hi],
                             4, c1, h1),
                        ):
                            sg = loop.tile([P, 8, CH], bf, tag="sg")
                            for m in range(8):
                                ps = psum.tile([P, CH], f32, tag="ps")
                                for h2 in range(NMM):
                                    lo = n * CH + h2 * MM
                                    for k in range(nk):
                                        nc.tensor.matmul(
                                            ps[:, h2 * MM:(h2 + 1) * MM],
                                            wt[:, k, m * P:(m + 1) * P],
                                            rhs_of_k(k, lo, lo + MM),
                                            start=(k == 0), stop=(k == nk - 1),
                                        )
                                func = AF.Tanh if m in (4, 5) else AF.Sigmoid
                                nc.scalar.activation(
                                    sg[:, m, :], ps[:], func, bias=bgt[:, m:m + 1]
                                )
                            # LSTM cell elementwise
                            nc.vector.tensor_tensor(
                                sg[:, 0:2, :], sg[:, 0:2, :], sg[:, 4:6, :], ALU.mult)
                            nc.vector.tensor_tensor(
                                cc[:, :, cs], sg[:, 2:4, :], cc[:, :, cs], ALU.mult)
                            nc.vector.tensor_tensor(
                                cc[:, :, cs], cc[:, :, cs], sg[:, 0:2, :], ALU.add)
                            nc.scalar.activation(sg[:, 4:6, :], cc[:, :, cs], AF.Tanh)
                            nc.vector.tensor_tensor(
                                hh[:, :, cs], sg[:, 6:8, :], sg[:, 4:6, :], ALU.mult)

                        # outcome modifier
                        if stage < 2:
                            continue
                        rom = loop.tile([P, 2, CH], bf, tag="rom")
                        for m in range(2):
                            ps = psum.tile([P, CH], f32, tag="ps")
                            for h2 in range(NMM):
                                lo = n * CH + h2 * MM
                                for k in range(3):
                                    rhs = (h1[:, k, lo:lo + MM] if k < 2
                                           else sbf[:, lo:lo + MM])
                                    nc.tensor.matmul(
                                        ps[:, h2 * MM:(h2 + 1) * MM],
                                        wt_om1[:, k, m * P:(m + 1) * P], rhs,
                                        start=(k == 0), stop=(k == 2),
                                    )
                            nc.vector.tensor_scalar(
                                rom[:, m, :], ps[:], bom1[:, m:m + 1], 0.0,
                                ALU.add, ALU.max)
                        mod = loop.tile([P, 4, CH], bf, tag="mod")
                        for m in range(4):
                            ps = psum.tile([P, CH], f32, tag="ps")
                            for h2 in range(NMM):
                                for k in range(2):
                                    nc.tensor.matmul(
                                        ps[:, h2 * MM:(h2 + 1) * MM],
                                        wt_om2[:, k, m * P:(m + 1) * P],
                                        rom[:, k, h2 * MM:(h2 + 1) * MM],
                                        start=(k == 0), stop=(k == 1),
                                    )
                            nc.scalar.activation(
                                mod[:, m, :], ps[:], AF.Tanh, bias=bom2[:, m:m + 1])
                        nc.vector.scalar_tensor_tensor(
                            cur[:, :, cs], mod[:], 0.1, cur[:, :, cs],
                            ALU.mult, ALU.add)
                        nc.sync.dma_start(
                            alt_d[t].rearrange("(fc p) r -> p fc r", p=P)[:, :, cs],
                            cur[:, :, cs],
                        )

                        # probability estimator: P += (0.1 W_pe1) @ mod
                        if stage < 3:
                            continue
                        ps = psum.tile([P, CH], f32, tag="ps")
                        for h2 in range(NMM):
                            for k in range(4):
                                nc.tensor.matmul(
                                    ps[:, h2 * MM:(h2 + 1) * MM],
                                    wt_pe1s[:, k, :],
                                    mod[:, k, h2 * MM:(h2 + 1) * MM],
                                    start=(k == 0), stop=(k == 3),
                                )
                        nc.vector.tensor_tensor(
                            pacc[:, cs], pacc[:, cs], ps[:], ALU.add)
                        rp = loop.tile([P, CH], bf, tag="rp")
                        nc.vector.tensor_scalar_max(rp[:], pacc[:, cs], 0.0)
                        if stage < 4:
                            continue
                        ps2 = psum.tile([P, CH], f32, tag="ps")
                        for h2 in range(NMM):
                            nc.tensor.matmul(
                                ps2[0:1, h2 * MM:(h2 + 1) * MM],
                                wt_pe2[:, 0:1],
                                rp[:, h2 * MM:(h2 + 1) * MM],
                                start=True, stop=True,
                            )
                        if stage < 5:
                            continue
                        pb = loop.tile([1, CH], f32, tag="pb")
                        nc.scalar.activation(
                            pb[0:1, :], ps2[0:1, :], AF.Sigmoid, bias=bpe2[0:1, 0:1])
                        if stage >= 6:
                            nc.sync.dma_start(
                                probs_d[t:t + 1, cs], pb[0:1, :])

    nc.compile()
    return nc


def _get_nc(num_steps: int):
    if num_steps not in _BUILD_CACHE:
        _BUILD_CACHE[num_steps] = _build(num_steps)
    return _BUILD_CACHE[num_steps]


def make_in_maps(inputs, R_=None):
    """Shard inputs + preprocess params into per-core input maps."""
    def npf(x):
        return np.asarray(x, np.float32)

    bf16 = ml_dtypes.bfloat16
    params = {
        "wt_g0": np.ascontiguousarray(
            np.concatenate([npf(inputs["w_hh0"]).T, npf(inputs["w_ih0"]).T], 0)
        ).astype(bf16),
        "wt_g1": np.ascontiguousarray(
            np.concatenate([npf(inputs["w_ih1"]).T, npf(inputs["w_hh1"]).T], 0)
        ).astype(bf16),
        "wt_om1": np.ascontiguousarray(npf(inputs["w_om1"]).T).astype(bf16),
        "wt_om2": np.ascontiguousarray(npf(inputs["w_om2"]).T).astype(bf16),
        "wt_se1": np.ascontiguousarray(npf(inputs["w_se1"]).T).astype(bf16),
        "wt_se2": np.ascontiguousarray(npf(inputs["w_se2"]).T).astype(bf16),
        "wt_pe1": np.ascontiguousarray(npf(inputs["w_pe1"]).T),
        "wt_pe1s": np.ascontiguousarray(0.1 * npf(inputs["w_pe1"]).T).astype(bf16),
        "wt_pe2": np.ascontiguousarray(npf(inputs["w_pe2"]).T).astype(bf16),
        "bg0": npf(inputs["b_ih0"]) + npf(inputs["b_hh0"]),
        "bg1": npf(inputs["b_ih1"]) + npf(inputs["b_hh1"]),
        "b_se1": npf(inputs["b_se1"]),
        "b_se2": npf(inputs["b_se2"]),
        "b_om1": npf(inputs["b_om1"]),
        "b_om2": npf(inputs["b_om2"]),
        "b_pe1": npf(inputs["b_pe1"]),
        "b_pe2": npf(inputs["b_pe2"]),
    }
    rr = R_ or R
    base = npf(inputs["base_timeline"])
    cf = npf(inputs["counterfactual_scenario"])
    ncores = base.shape[0] // rr
    in_maps = []
    for i in range(ncores):
        m = dict(params)
        m["base"] = np.ascontiguousarray(base[i * rr:(i + 1) * rr])
        m["cf"] = np.ascontiguousarray(cf[i * rr:(i + 1) * rr])
        in_maps.append(m)
    return in_maps


def assemble(results):
    """Gather per-core transposed outputs into full reference-shaped arrays."""
    alt = np.concatenate(
        [r["alt_t"].transpose(2, 0, 1) for r in results], axis=0)      # [B, T, 512]
    probs = np.concatenate(
        [r["probs_t"].T[:, :, None] for r in results], axis=0)         # [B, T, 1]
    s = np.concatenate([r["s_t"].T for r in results], axis=0)          # [B, 128]
    final = np.ascontiguousarray(alt[:, -1, :])                        # [B, 512]
    return alt, probs, s, final


def kernel(**inputs):
    from concourse.bass_utils import run_bass_kernel_spmd

    num_steps = int(np.asarray(inputs["num_steps"]))
    nc = _get_nc(num_steps)
    in_maps = make_in_maps(inputs)
    res = run_bass_kernel_spmd(nc, in_maps, core_ids=list(range(NCORES))).results
    return assemble(res)


# revision 17
# speedup vs baseline: 1.8236x; 1.7213x over previous
"""AlternateTimelineGenerator Trainium2 kernel.

Data-parallel over the batch: 16384 rows -> 8 NeuronCores x 2048 rows.
Per core, everything runs in "transposed" layout (feature on SBUF
partitions, batch rows on the free dimension) so the whole chain of
matmuls composes without on-chip transposes; only the two inputs are
transposed once at the start via TensorE. Outputs are written transposed
and de-transposed on the host during the gather.

Per step (rows chunked in CH-column slices):
  gates0 = W_hh0 @ h0 + W_ih0 @ s            (PE bf16, PSUM fp32)
  i,f,g,o = ACT sigmoid/tanh (+bias)          -> bf16 SBUF
  c0 = f*c0 + i*g; h0 = o*tanh(c0)            (DVE, c fp32 / h bf16)
  gates1 = W_ih1 @ h0 + W_hh1 @ h1            ... same cell for layer 1
  rom  = relu(W_om1 @ [h1; s])                (PE + DVE relu, bf16)
  mod  = tanh(W_om2 @ rom + b)                (PE + ACT, bf16)
  cur += 0.1 * mod                            (DVE scalar_tensor_tensor)
  P   += (0.1*W_pe1) @ mod                    (PE bf16; P = W_pe1@cur + b)
  prob = sigmoid(W_pe2 @ relu(P))             (PE bf16 + DVE + ACT)

The probability head keeps P = W_pe1 @ cur^T + b_pe1 as a recurrent
fp32 state so it never has to read the freshly-updated cur (which
would serialize PE behind the big DVE update each step).
"""

import numpy as np
import ml_dtypes

B, FDIM, H, E = 16384, 512, 256, 128
NCORES = 8
R = B // NCORES        # rows per core
CH = 512               # rows per chunk
P = 128

_BUILD_CACHE = {}


def _build(num_steps: int, R: int = R, CH: int = CH, stage: int = 99):
    import concourse.bass as bass
    import concourse.tile as tile
    from concourse import bacc, mybir
    from concourse.masks import make_identity

    NCH = R // CH
    MM = min(CH, 512)  # matmul moving-dim tile
    NMM = CH // MM
    dt = mybir.dt
    AF = mybir.ActivationFunctionType
    ALU = mybir.AluOpType
    f32, bf = dt.float32, dt.float16

    nc = bacc.Bacc("TRN2", target_bir_lowering=False, debug=False)

    # ---- DRAM I/O ----
    base_d = nc.dram_tensor("base", [R, FDIM], f32, kind="ExternalInput").ap()
    cf_d = nc.dram_tensor("cf", [R, FDIM], f32, kind="ExternalInput").ap()
    wt_g0_d = nc.dram_tensor("wt_g0", [3 * P, 4 * H], bf, kind="ExternalInput").ap()
    wt_g1_d = nc.dram_tensor("wt_g1", [4 * P, 4 * H], bf, kind="ExternalInput").ap()
    wt_om1_d = nc.dram_tensor("wt_om1", [3 * P, 256], bf, kind="ExternalInput").ap()
    wt_om2_d = nc.dram_tensor("wt_om2", [256, FDIM], bf, kind="ExternalInput").ap()
    wt_se1_d = nc.dram_tensor("wt_se1", [FDIM, 256], bf, kind="ExternalInput").ap()
    wt_se2_d = nc.dram_tensor("wt_se2", [256, E], bf, kind="ExternalInput").ap()
    wt_pe1_d = nc.dram_tensor("wt_pe1", [FDIM, E], f32, kind="ExternalInput").ap()
    wt_pe1s_d = nc.dram_tensor("wt_pe1s", [FDIM, E], bf, kind="ExternalInput").ap()
    wt_pe2_d = nc.dram_tensor("wt_pe2", [E, 1], bf, kind="ExternalInput").ap()
    bg0_d = nc.dram_tensor("bg0", [4 * H], f32, kind="ExternalInput").ap()
    bg1_d = nc.dram_tensor("bg1", [4 * H], f32, kind="ExternalInput").ap()
    bse1_d = nc.dram_tensor("b_se1", [256], f32, kind="ExternalInput").ap()
    bse2_d = nc.dram_tensor("b_se2", [E], f32, kind="ExternalInput").ap()
    bom1_d = nc.dram_tensor("b_om1", [256], f32, kind="ExternalInput").ap()
    bom2_d = nc.dram_tensor("b_om2", [FDIM], f32, kind="ExternalInput").ap()
    bpe1_d = nc.dram_tensor("b_pe1", [E], f32, kind="ExternalInput").ap()

    alt_d = nc.dram_tensor("alt_t", [num_steps, FDIM, R], f32, kind="ExternalOutput").ap()
    probs_d = nc.dram_tensor("probs_t", [num_steps, R], f32, kind="ExternalOutput").ap()
    s_d = nc.dram_tensor("s_t", [E, R], f32, kind="ExternalOutput").ap()

    with tile.TileContext(nc) as tc:
        with (
            tc.tile_pool(name="consts", bufs=1) as consts,
            tc.tile_pool(name="state", bufs=1) as state,
            tc.tile_pool(name="psum", bufs=5, space="PSUM") as psum,
            tc.tile_pool(name="psumx", bufs=3, space="PSUM") as psumx,
        ):
            # ---- weights / biases to SBUF ----
            def w_tile(name, dram, kk, mm_, dtype):
                tl = consts.tile([P, kk, mm_], dtype, tag=name)
                nc.sync.dma_start(tl[:], dram.rearrange("(k p) m -> p k m", p=P))
                return tl

            wt_g0 = w_tile("wt_g0", wt_g0_d, 3, 4 * H, bf)
            wt_g1 = w_tile("wt_g1", wt_g1_d, 4, 4 * H, bf)
            wt_om1 = w_tile("wt_om1", wt_om1_d, 3, 256, bf)
            wt_om2 = w_tile("wt_om2", wt_om2_d, 2, FDIM, bf)
            wt_se1 = w_tile("wt_se1", wt_se1_d, 4, 256, bf)
            wt_se2 = w_tile("wt_se2", wt_se2_d, 2, E, bf)
            wt_pe1 = w_tile("wt_pe1", wt_pe1_d, 4, E, f32)
            wt_pe1s = w_tile("wt_pe1s", wt_pe1s_d, 4, E, bf)
            wt_pe2 = consts.tile([P, 1], bf, tag="wt_pe2")
            nc.sync.dma_start(wt_pe2[:], wt_pe2_d)

            def bias_tile(name, dram, m):
                tl = consts.tile([P, m], f32, tag=name)
                nc.sync.dma_start(tl[:], dram.rearrange("(m p) -> p m", p=P))
                return tl

            bg0 = bias_tile("bg0", bg0_d, 8)
            bg1 = bias_tile("bg1", bg1_d, 8)
            bse1 = bias_tile("bse1", bse1_d, 2)
            bse2 = bias_tile("bse2", bse2_d, 1)
            bom1 = bias_tile("bom1", bom1_d, 2)
            bom2 = bias_tile("bom2", bom2_d, 4)
            bpe1 = bias_tile("bpe1", bpe1_d, 1)

            ident = consts.tile([P, P], f32, tag="ident")
            make_identity(nc, ident[:])

            # ---- persistent state ----
            h0 = state.tile([P, 2, R], bf, tag="h0")
            h1 = state.tile([P, 2, R], bf, tag="h1")
            c0 = state.tile([P, 2, R], f32, tag="c0")
            c1 = state.tile([P, 2, R], f32, tag="c1")
            cur = state.tile([P, 4, R], f32, tag="cur")
            pacc = state.tile([P, R], f32, tag="pacc")
            s = state.tile([P, R], f32, tag="s")
            sbf = state.tile([P, R], bf, tag="sbf")
            for tl in (h0, h1, c0, c1):
                nc.vector.memset(tl[:], 0.0)

            # ---- transpose inputs (cf -> cfT bf16, base -> cur fp32) ----
            with tc.tile_pool(name="setup", bufs=3) as setup, \
                 tc.tile_pool(name="setup1", bufs=1) as setup1:
                cfT = setup1.tile([P, 4, R], bf, tag="cfT")
                r1 = setup1.tile([P, 2, R], bf, tag="r1")
                for src, dst in ((cf_d, cfT), (base_d, cur)):
                    for rt in range(R // P):
                        tmp = setup.tile([P, FDIM], f32, tag="tr_in")
                        nc.sync.dma_start(tmp[:], src[rt * P:(rt + 1) * P, :])
                        pt = psum.tile([P, 4, P], f32, tag="ps")
                        for ft in range(4):
                            nc.tensor.transpose(
                                pt[:, ft, :], tmp[:, ft * P:(ft + 1) * P], ident[:]
                            )
                        nc.vector.tensor_copy(dst[:, :, rt * P:(rt + 1) * P], pt[:])

                # ---- scenario encoder (bf16) + P0 = W_pe1 @ base^T + b ----
                for n in range(R // 512):
                    sl = slice(n * 512, (n + 1) * 512)
                    for m in range(2):
                        ps = psum.tile([P, 512], f32, tag="ps")
                        for k in range(4):
                            nc.tensor.matmul(
                                ps[:], wt_se1[:, k, m * P:(m + 1) * P],
                                cfT[:, k, sl], start=(k == 0), stop=(k == 3),
                            )
                        nc.scalar.activation(
                            r1[:, m, sl], ps[:], AF.Relu, bias=bse1[:, m:m + 1])
                    ps = psum.tile([P, 512], f32, tag="ps")
                    for k in range(2):
                        nc.tensor.matmul(
                            ps[:], wt_se2[:, k, :], r1[:, k, sl],
                            start=(k == 0), stop=(k == 1),
                        )
                    nc.scalar.activation(
                        s[:, sl], ps[:], AF.Identity, bias=bse2[:, 0:1])
                    ps = psum.tile([P, 512], f32, tag="ps")
                    for k in range(4):
                        nc.tensor.matmul(
                            ps[:], wt_pe1[:, k, :], cur[:, k, sl],
                            start=(k == 0), stop=(k == 3),
                        )
                    nc.vector.tensor_scalar(
                        pacc[:, sl], ps[:], bpe1[:, 0:1], None, ALU.add)
                nc.vector.tensor_copy(sbf[:], s[:])
                nc.sync.dma_start(s_d[:, :], s[:])

            # ---- recurrence (layer-major across chunks) ----
            with tc.tile_pool(name="loop", bufs=2) as loop:
                for t in range(num_steps):
                    sgs = {}
                    for cell, (wt, bgt, nk, cc, hh) in enumerate((
                        (wt_g0, bg0, 3, c0, h0),
                        (wt_g1, bg1, 4, c1, h1),
                    )):
                        for n in range(NCH):
                            cs = slice(n * CH, (n + 1) * CH)
                            sg = loop.tile([P, 8, CH], bf, tag="sg", bufs=5)
                            sgs[n] = sg
                            for m in range(8):
                                ps = psum.tile([P, CH], f32, tag="ps")
                                for h2 in range(NMM):
                                    lo = n * CH + h2 * MM
                                    for k in range(nk):
                                        if cell == 0:
                                            rhs = (h0[:, k, lo:lo + MM] if k < 2
                                                   else sbf[:, lo:lo + MM])
                                        else:
                                            rhs = (h0[:, k, lo:lo + MM] if k < 2
                                                   else h1[:, k - 2, lo:lo + MM])
                                        nc.tensor.matmul(
                                            ps[:, h2 * MM:(h2 + 1) * MM],
                                            wt[:, k, m * P:(m + 1) * P], rhs,
                                            start=(k == 0), stop=(k == nk - 1),
                                        )
                                func = AF.Tanh if m in (4, 5) else AF.Sigmoid
                                nc.scalar.activation(
                                    sg[:, m, :], ps[:], func, bias=bgt[:, m:m + 1])
                        for n in range(NCH):
                            cs = slice(n * CH, (n + 1) * CH)
                            sg = sgs[n]
                            nc.vector.tensor_tensor(
                                sg[:, 0:2, :], sg[:, 0:2, :], sg[:, 4:6, :], ALU.mult)
                            nc.vector.tensor_tensor(
                                cc[:, :, cs], sg[:, 2:4, :], cc[:, :, cs], ALU.mult)
                            nc.vector.tensor_tensor(
                                cc[:, :, cs], cc[:, :, cs], sg[:, 0:2, :], ALU.add)
                            nc.scalar.activation(sg[:, 4:6, :], cc[:, :, cs], AF.Tanh)
                            nc.vector.tensor_tensor(
                                hh[:, :, cs], sg[:, 6:8, :], sg[:, 4:6, :], ALU.mult)

                    roms = {}
                    for n in range(NCH):
                        rom = loop.tile([P, 2, CH], bf, tag="rom", bufs=3)
                        roms[n] = rom
                        for m in range(2):
                            ps = psumx.tile([P, CH], f32, tag="px")
                            for h2 in range(NMM):
                                lo = n * CH + h2 * MM
                                for k in range(3):
                                    rhs = (h1[:, k, lo:lo + MM] if k < 2
                                           else sbf[:, lo:lo + MM])
                                    nc.tensor.matmul(
                                        ps[:, h2 * MM:(h2 + 1) * MM],
                                        wt_om1[:, k, m * P:(m + 1) * P], rhs,
                                        start=(k == 0), stop=(k == 2),
                                    )
                            nc.vector.tensor_scalar(
                                rom[:, m, :], ps[:], bom1[:, m:m + 1], 0.0,
                                ALU.add, ALU.max)
                    mods = {}
                    for n in range(NCH):
                        cs = slice(n * CH, (n + 1) * CH)
                        rom = roms[n]
                        mod = loop.tile([P, 4, CH], bf, tag="mod", bufs=4)
                        mods[n] = mod
                        for m in range(4):
                            ps = psumx.tile([P, CH], f32, tag="px")
                            for h2 in range(NMM):
                                for k in range(2):
                                    nc.tensor.matmul(
                                        ps[:, h2 * MM:(h2 + 1) * MM],
                                        wt_om2[:, k, m * P:(m + 1) * P],
                                        rom[:, k, h2 * MM:(h2 + 1) * MM],
                                        start=(k == 0), stop=(k == 1),
                                    )
                            nc.scalar.activation(
                                mod[:, m, :], ps[:], AF.Tanh, bias=bom2[:, m:m + 1])
                        nc.vector.scalar_tensor_tensor(
                            cur[:, :, cs], mod[:], 0.1, cur[:, :, cs],
                            ALU.mult, ALU.add)
                        nc.sync.dma_start(
                            alt_d[t].rearrange("(fc p) r -> p fc r", p=P)[:, :, cs],
                            cur[:, :, cs],
                        )
                    for n in range(NCH):
                        cs = slice(n * CH, (n + 1) * CH)
                        mod = mods[n]
                        ps = psumx.tile([P, CH], f32, tag="px")
                        for h2 in range(NMM):
                            for k in range(4):
                                nc.tensor.matmul(
                                    ps[:, h2 * MM:(h2 + 1) * MM],
                                    wt_pe1s[:, k, :],
                                    mod[:, k, h2 * MM:(h2 + 1) * MM],
                                    start=(k == 0), stop=(k == 3),
                                )
                        nc.vector.tensor_tensor(
                            pacc[:, cs], pacc[:, cs], ps[:], ALU.add)
                        rp = loop.tile([P, CH], bf, tag="rp", bufs=2)
                        nc.vector.tensor_scalar_max(rp[:], pacc[:, cs], 0.0)
                        ps2 = psumx.tile([P, CH], f32, tag="px")
                        for h2 in range(NMM):
                            nc.tensor.matmul(
                                ps2[0:1, h2 * MM:(h2 + 1) * MM],
                                wt_pe2[:, 0:1],
                                rp[:, h2 * MM:(h2 + 1) * MM],
                                start=True, stop=True,
                            )
                        pb = loop.tile([1, CH], f32, tag="pb", bufs=2)
                        nc.vector.tensor_copy(pb[0:1, :], ps2[0:1, :])
                        nc.sync.dma_start(
                            probs_d[t:t + 1, cs], pb[0:1, :])

    nc.compile()
    return nc


def _get_nc(num_steps: int):
    if num_steps not in _BUILD_CACHE:
        _BUILD_CACHE[num_steps] = _build(num_steps)
    return _BUILD_CACHE[num_steps]


def make_in_maps(inputs, R_=None):
    """Shard inputs + preprocess params into per-core input maps."""
    def npf(x):
        return np.asarray(x, np.float32)

    bf16 = np.float16
    params = {
        "wt_g0": np.ascontiguousarray(
            np.concatenate([npf(inputs["w_hh0"]).T, npf(inputs["w_ih0"]).T], 0)
        ).astype(bf16),
        "wt_g1": np.ascontiguousarray(
            np.concatenate([npf(inputs["w_ih1"]).T, npf(inputs["w_hh1"]).T], 0)
        ).astype(bf16),
        "wt_om1": np.ascontiguousarray(npf(inputs["w_om1"]).T).astype(bf16),
        "wt_om2": np.ascontiguousarray(npf(inputs["w_om2"]).T).astype(bf16),
        "wt_se1": np.ascontiguousarray(npf(inputs["w_se1"]).T).astype(bf16),
        "wt_se2": np.ascontiguousarray(npf(inputs["w_se2"]).T).astype(bf16),
        "wt_pe1": np.ascontiguousarray(npf(inputs["w_pe1"]).T),
        "wt_pe1s": np.ascontiguousarray(0.1 * npf(inputs["w_pe1"]).T).astype(bf16),
        "wt_pe2": np.ascontiguousarray(npf(inputs["w_pe2"]).T).astype(bf16),
        "bg0": npf(inputs["b_ih0"]) + npf(inputs["b_hh0"]),
        "bg1": npf(inputs["b_ih1"]) + npf(inputs["b_hh1"]),
        "b_se1": npf(inputs["b_se1"]),
        "b_se2": npf(inputs["b_se2"]),
        "b_om1": npf(inputs["b_om1"]),
        "b_om2": npf(inputs["b_om2"]),
        "b_pe1": npf(inputs["b_pe1"]),
    }
    rr = R_ or R
    base = npf(inputs["base_timeline"])
    cf = npf(inputs["counterfactual_scenario"])
    ncores = base.shape[0] // rr
    in_maps = []
    for i in range(ncores):
        m = dict(params)
        m["base"] = np.ascontiguousarray(base[i * rr:(i + 1) * rr])
        m["cf"] = np.ascontiguousarray(cf[i * rr:(i + 1) * rr])
        in_maps.append(m)
    return in_maps


def assemble(results, b_pe2=0.0):
    """Gather per-core transposed outputs into full reference-shaped arrays."""
    alt = np.concatenate(
        [r["alt_t"].transpose(2, 0, 1) for r in results], axis=0)      # [B, T, 512]
    logits = np.concatenate(
        [r["probs_t"].T[:, :, None] for r in results], axis=0)         # [B, T, 1]
    x = logits.astype(np.float64) + float(np.asarray(b_pe2).reshape(-1)[0])
    probs = (1.0 / (1.0 + np.exp(-x))).astype(np.float32)
    s = np.concatenate([r["s_t"].T for r in results], axis=0)          # [B, 128]
    final = np.ascontiguousarray(alt[:, -1, :])                        # [B, 512]
    return alt, probs, s, final


def kernel(**inputs):
    from concourse.bass_utils import run_bass_kernel_spmd

    num_steps = int(np.asarray(inputs["num_steps"]))
    nc = _get_nc(num_steps)
    in_maps = make_in_maps(inputs)
    res = run_bass_kernel_spmd(nc, in_maps, core_ids=list(range(NCORES))).results
    return assemble(res, inputs["b_pe2"])
